# revision 15
# baseline (speedup 1.0000x reference)
"""Bahdanau attention (nn_BauAttn) Trainium2 Bass kernel.

Data-parallel over batch: 32 batches -> 8 cores x 4 batches. Each core:
  keys   = enc[:, b, :] @ W_enc + (hidden[b] @ W_hidden + b_attn)   [2048, 1024]
  align  = mask + tanh(keys) . v                                     [2048]
  p      = softmax(align)  (over the 2048 time steps)
  ctx    = p . enc[:, b, :]                                          [1024]

Main matmul runs on the PE with fp32r (1 cycle/row). enc tiles are
transposed on the PE (fp32 transpose mode) so the contraction dim (e)
lands on partitions. The query bias is folded into the PSUM accumulation
group as a K=1 ones-row matmul. tanh on ACT; the v-dot is a fused
DVE tensor_tensor_reduce with the mask as the reduction init. Softmax
skips the max-shift (additive logits are O(10); exp is safely finite in
fp32) so the exp->context chain stays short. Context matvecs reuse the
SBUF-resident enc tiles of the current batch.
"""

import os
import sys

for _p in ("/root/.axon_site", "/root/.axon_site/_ro/trn_rl_repo",
           "/root/.axon_site/_ro/pypackages"):
    if os.path.isdir(_p) and _p not in sys.path:
        sys.path.append(_p)

import numpy as np

import concourse.bass as bass
import concourse.mybir as mybir
import concourse.tile as tile
from concourse import bacc
from concourse.bass_utils import run_bass_kernel_spmd
from concourse.masks import make_identity

F32 = mybir.dt.float32
F32R = mybir.dt.float32r

P = 128
L, B, E, A, H = 2048, 32, 1024, 1024, 1024
NCORES = 8
BC = B // NCORES          # batches per core = 4
KC = E // P               # contraction chunks = 8
NLB = L // P              # l-blocks per batch = 16
NH = A // 512             # 512-wide output halves = 2

AF = mybir.ActivationFunctionType
ALU = mybir.AluOpType


def build_nc(finalize=True) -> bass.Bass:
    nc = bacc.Bacc(target_bir_lowering=False)

    enc = nc.declare_dram_parameter("enc_outputs", [L, BC, E], F32, isOutput=False)
    mask = nc.declare_dram_parameter("mask", [L, BC], F32, isOutput=False)
    hid = nc.declare_dram_parameter("hidden_state", [BC, H], F32, isOutput=False)
    w_enc = nc.declare_dram_parameter("W_enc", [E, A], F32, isOutput=False)
    b_attn = nc.declare_dram_parameter("b_attn", [A], F32, isOutput=False)
    w_hid = nc.declare_dram_parameter("W_hidden", [H, A], F32, isOutput=False)
    v = nc.declare_dram_parameter("v", [A], F32, isOutput=False)
    ctx_out = nc.declare_dram_parameter("context", [BC, E], F32, isOutput=True)
    align_out = nc.declare_dram_parameter("alignment", [L, BC], F32, isOutput=True)

    with tile.TileContext(nc) as tc:
        with tc.tile_pool(name="const", bufs=1) as const:
            ident = const.tile([P, P], F32)
            make_identity(nc, ident)
            ones_f32 = const.tile([1, P], F32)
            nc.gpsimd.memset(ones_f32, 1.0)
            ones_r = const.tile([1, P], F32R)
            nc.sync.dma_start(ones_r, ones_f32.bitcast(F32R))

            w_sb = const.tile([P, KC, A], F32R)
            nc.sync.dma_start(
                w_sb, w_enc.rearrange("(k p) a -> p k a", p=P).bitcast(F32R)
            )

            v_row = const.tile([1, A], F32)
            nc.sync.dma_start(v_row, v[None, :])
            v_bc = const.tile([P, A], F32)

            battn4 = const.tile([BC, A], F32)
            for b in range(BC):
                nc.sync.dma_start(battn4[b : b + 1, :], b_attn[None, :])

            mask_sb = const.tile([P, NLB, BC], F32)
            nc.sync.dma_start(mask_sb, mask.rearrange("(i p) b -> p i b", p=P))

            align_slab = const.tile([P, NLB, BC], F32)
            p_slab = const.tile([P, NLB, BC], F32)
            qb4 = const.tile([BC, A], F32)
            qb_sb = const.tile([1, BC, A], F32R)  # q+b_attn on partition 0
            ctx_sb = const.tile([1, BC, E], F32)  # partition 0

            # ---- q = hidden @ W_hidden + b_attn  (tiny, one-time) ----
            with (
                tc.tile_pool(name="qtmp", bufs=2) as qtmp,
                tc.tile_pool(name="qps", bufs=2, space="PSUM") as qps,
                tc.tile_pool(name="qtps", bufs=2, space="PSUM") as qtps,
            ):
                for h in range(NH):
                    vb_ps = qps.tile([P, 512], F32, tag="vb", name=f"vb_ps{h}")
                    nc.tensor.matmul(
                        vb_ps,
                        lhsT=ones_f32,
                        rhs=v_row[:, h * 512 : (h + 1) * 512],
                        start=True,
                        stop=True,
                    )
                    nc.vector.tensor_copy(v_bc[:, h * 512 : (h + 1) * 512], vb_ps)
                hid_sb = qtmp.tile([BC, H], F32, tag="hid")
                nc.sync.dma_start(hid_sb, hid[:, :])
                hidT = qtmp.tile([P, KC, BC], F32R, tag="hidT")
                for k in range(KC):
                    tp = qtps.tile([P, BC], F32, tag="qtp")
                    nc.tensor.transpose(
                        tp, hid_sb[:, k * P : (k + 1) * P], ident[:BC, :BC]
                    )
                    nc.vector.tensor_copy(hidT[:, k, :], tp)
                q_ps = [qps.tile([BC, 512], F32, tag="qps", name=f"q_ps{h}") for h in range(NH)]
                for k in range(KC):
                    wh = qtmp.tile([P, A], F32R, tag="wh")
                    nc.sync.dma_start(
                        wh, w_hid[k * P : (k + 1) * P, :].bitcast(F32R)
                    )
                    for h in range(NH):
                        nc.tensor.matmul(
                            q_ps[h],
                            lhsT=hidT[:, k, :],
                            rhs=wh[:, h * 512 : (h + 1) * 512],
                            start=(k == 0),
                            stop=(k == KC - 1),
                        )
                for h in range(NH):
                    sl = slice(h * 512, (h + 1) * 512)
                    nc.vector.tensor_add(qb4[:, sl], q_ps[h], battn4[:, sl])
                for b in range(BC):
                    nc.sync.dma_start(
                        qb_sb[0:1, b, :], qb4[b : b + 1, :].bitcast(F32R)
                    )

            # ---- main pipeline ----
            with (
                tc.tile_pool(name="xpool", bufs=20) as xpool,
                tc.tile_pool(name="xtpool", bufs=3) as xtpool,
                tc.tile_pool(name="thpool", bufs=2) as thpool,
                tc.tile_pool(name="scrpool", bufs=2) as scrpool,
                tc.tile_pool(name="smpool", bufs=2) as smpool,
                tc.tile_pool(name="eslab", bufs=2) as espool,
                tc.tile_pool(name="keysps", bufs=2, space="PSUM") as keysps,
                tc.tile_pool(name="tpps", bufs=2, space="PSUM") as tpps,
                tc.tile_pool(name="ctxps", bufs=2, space="PSUM") as ctxps,
                tc.tile_pool(name="tinyps", bufs=1, space="PSUM") as tinyps,
            ):
                x_tiles = {}

                def emit_block(b, i):
                    x = xpool.tile([P, E], F32R, tag="x", name=f"x_{b}_{i}")
                    nc.sync.dma_start(
                        x, enc[i * P : (i + 1) * P, b, :].bitcast(F32R)
                    )
                    x_tiles[(b, i)] = x

                    xt = xtpool.tile([P, E], F32R, tag="xt", name=f"xt_{b}_{i}")
                    for g in range(2):
                        tp = tpps.tile([P, 512], F32, tag="tp", name=f"tp_{b}_{i}_{g}")
                        for j in range(4):
                            k = g * 4 + j
                            nc.tensor.transpose(
                                tp[:, j * P : (j + 1) * P],
                                x[:, k * P : (k + 1) * P].bitcast(F32),
                                ident,
                            )
                        nc.vector.tensor_copy(xt[:, g * 512 : (g + 1) * 512], tp)

                    th = thpool.tile([P, E], F32, tag="th", name=f"th_{b}_{i}")
                    kp = [
                        keysps.tile([P, 512], F32, tag="kp", name=f"kp_{b}_{i}_{h}")
                        for h in range(NH)
                    ]
                    for h in range(NH):
                        nc.tensor.matmul(
                            kp[h],
                            lhsT=ones_r,
                            rhs=qb_sb[0:1, b, h * 512 : (h + 1) * 512],
                            start=True,
                            stop=False,
                        )
                    for k in range(KC):
                        for h in range(NH):
                            nc.tensor.matmul(
                                kp[h],
                                lhsT=xt[:, k * P : (k + 1) * P],
                                rhs=w_sb[:, k, h * 512 : (h + 1) * 512],
                                start=False,
                                stop=(k == KC - 1),
                            )
                    for h in range(NH):
                        nc.scalar.activation(
                            th[:, h * 512 : (h + 1) * 512], kp[h], AF.Tanh
                        )

                    scr = scrpool.tile([P, E], F32, tag="scr", name=f"scr_{b}_{i}")
                    nc.vector.tensor_mul(scr, th, v_bc)
                    nc.vector.tensor_reduce(
                        align_slab[:, i, b : b + 1],
                        scr,
                        axis=mybir.AxisListType.X,
                        op=ALU.add,
                    )

                def emit_epilogue(b):
                    # exp (no max-shift; logits are O(10)) + per-partition sum
                    e_in = espool.tile([P, NLB], F32, tag="ein", name=f"ein_{b}")
                    nc.vector.tensor_add(e_in, align_slab[:, :, b], mask_sb[:, :, b])
                    e_sl = espool.tile([P, NLB], F32, tag="esl", name=f"esl_{b}")
                    rowsum = smpool.tile([P, 1], F32, tag="rs", name=f"rs_{b}")
                    nc.scalar.activation(e_sl, e_in, AF.Exp, accum_out=rowsum)
                    e_slr = espool.tile([P, NLB], F32R, tag="eslr", name=f"eslr_{b}")
                    nc.vector.tensor_copy(e_slr, e_sl)
                    # context matvecs on unnormalized exp weights
                    for h in range(NH):
                        cp = ctxps.tile([1, 512], F32, tag="cp", name=f"cp_{b}_{h}")
                        for i in range(NLB):
                            nc.tensor.matmul(
                                cp,
                                lhsT=e_slr[:, i : i + 1],
                                rhs=x_tiles[(b, i)][:, h * 512 : (h + 1) * 512],
                                start=(i == 0),
                                stop=(i == NLB - 1),
                            )
                        # normalization: partition-sum of rowsum via PE transpose
                        if h == 0:
                            ts = tinyps.tile([1, P], F32, tag="ts", name=f"ts_{b}")
                            nc.tensor.transpose(ts, rowsum, ident)
                            s_sb = smpool.tile([1, 1], F32, tag="s", name=f"s_{b}")
                            nc.vector.tensor_reduce(
                                s_sb, ts, axis=mybir.AxisListType.X, op=ALU.add
                            )
                            rinv = smpool.tile([1, 1], F32, tag="ri", name=f"ri_{b}")
                            nc.vector.reciprocal(rinv, s_sb)
                            rb_ps = tinyps.tile(
                                [P, 1], F32, tag="rbps", name=f"rbps_{b}"
                            )
                            nc.tensor.matmul(
                                rb_ps,
                                lhsT=ones_f32,
                                rhs=rinv,
                                start=True,
                                stop=True,
                            )
                            rinv_bc = smpool.tile(
                                [P, 1], F32, tag="rib", name=f"rib_{b}"
                            )
                            nc.vector.tensor_copy(rinv_bc, rb_ps)
                            nc.scalar.mul(p_slab[:, :, b], e_sl, rinv_bc)
                        nc.scalar.mul(
                            ctx_sb[0:1, b, h * 512 : (h + 1) * 512], cp, rinv
                        )

                for b in range(BC):
                    for i in range(NLB):
                        emit_block(b, i)
                        if b > 0 and i == 1:
                            emit_epilogue(b - 1)
                emit_epilogue(BC - 1)

                nc.sync.dma_start(
                    ctx_out[None, :, :], ctx_sb[0:1, :, :]
                )
                nc.sync.dma_start(
                    align_out.rearrange("(i p) b -> p i b", p=P), p_slab
                )

    if finalize:
        nc.finalize()
    return nc


_NC_CACHE = None


def _get_nc():
    global _NC_CACHE
    if _NC_CACHE is None:
        _NC_CACHE = build_nc()
    return _NC_CACHE


def _shard_inputs(inputs):
    enc = np.ascontiguousarray(np.asarray(inputs["enc_outputs"], dtype=np.float32))
    mask = np.ascontiguousarray(np.asarray(inputs["mask"], dtype=np.float32))
    hid = np.ascontiguousarray(np.asarray(inputs["hidden_state"], dtype=np.float32))
    w_enc = np.ascontiguousarray(np.asarray(inputs["W_enc"], dtype=np.float32))
    b_attn = np.ascontiguousarray(np.asarray(inputs["b_attn"], dtype=np.float32))
    w_hid = np.ascontiguousarray(np.asarray(inputs["W_hidden"], dtype=np.float32))
    v = np.ascontiguousarray(np.asarray(inputs["v"], dtype=np.float32))
    in_maps = []
    for c in range(NCORES):
        bs = slice(BC * c, BC * (c + 1))
        in_maps.append(
            {
                "enc_outputs": np.ascontiguousarray(enc[:, bs, :]),
                "mask": np.ascontiguousarray(mask[:, bs]),
                "hidden_state": np.ascontiguousarray(hid[bs, :]),
                "W_enc": w_enc,
                "b_attn": b_attn,
                "W_hidden": w_hid,
                "v": v,
            }
        )
    return in_maps


def run_spmd(inputs, trace=False):
    """Returns (context, alignment), BassKernelResults."""
    in_maps = _shard_inputs(inputs)
    res = run_bass_kernel_spmd(
        _get_nc(), in_maps, list(range(NCORES)), trace=trace
    )
    ctx = np.concatenate(
        [np.asarray(res.results[c]["context"]) for c in range(NCORES)], axis=0
    )
    align = np.concatenate(
        [np.asarray(res.results[c]["alignment"]) for c in range(NCORES)], axis=1
    )
    return (ctx, align), res


def kernel(**inputs):
    out, _ = run_spmd(inputs, trace=False)
    return out


# revision 20
# speedup vs baseline: 1.0074x; 1.0074x over previous
"""Bahdanau attention (nn_BauAttn) Trainium2 Bass kernel.

Data-parallel over batch: 32 batches -> 8 cores x 4 batches. Each core:
  keys   = enc[:, b, :] @ W_enc + (hidden[b] @ W_hidden + b_attn)   [2048, 1024]
  align  = mask + tanh(keys) . v                                     [2048]
  p      = softmax(align)  (over the 2048 time steps)
  ctx    = p . enc[:, b, :]                                          [1024]

Main matmul runs on the PE with fp32r (1 cycle/row). enc tiles are
transposed on the PE (fp32 transpose mode) so the contraction dim (e)
lands on partitions. The query bias is folded into the PSUM accumulation
group as a K=1 ones-row matmul. tanh on ACT; the v-dot is a fused
DVE tensor_tensor_reduce with the mask as the reduction init. Softmax
skips the max-shift (additive logits are O(10); exp is safely finite in
fp32) so the exp->context chain stays short. Context matvecs reuse the
SBUF-resident enc tiles of the current batch.
"""

import os
import sys

for _p in ("/root/.axon_site", "/root/.axon_site/_ro/trn_rl_repo",
           "/root/.axon_site/_ro/pypackages"):
    if os.path.isdir(_p) and _p not in sys.path:
        sys.path.append(_p)

import numpy as np

import concourse.bass as bass
import concourse.mybir as mybir
import concourse.tile as tile
from concourse import bacc
from concourse.bass_utils import run_bass_kernel_spmd
from concourse.masks import make_identity

F32 = mybir.dt.float32
F32R = mybir.dt.float32r

P = 128
L, B, E, A, H = 2048, 32, 1024, 1024, 1024
NCORES = 8
BC = B // NCORES          # batches per core = 4
KC = E // P               # contraction chunks = 8
NLB = L // P              # l-blocks per batch = 16
NH = A // 512             # 512-wide output halves = 2

AF = mybir.ActivationFunctionType
ALU = mybir.AluOpType


def build_nc(finalize=True) -> bass.Bass:
    nc = bacc.Bacc(target_bir_lowering=False)

    enc = nc.declare_dram_parameter("enc_outputs", [L, BC, E], F32, isOutput=False)
    mask = nc.declare_dram_parameter("mask", [L, BC], F32, isOutput=False)
    hid = nc.declare_dram_parameter("hidden_state", [BC, H], F32, isOutput=False)
    w_enc = nc.declare_dram_parameter("W_enc", [E, A], F32, isOutput=False)
    b_attn = nc.declare_dram_parameter("b_attn", [A], F32, isOutput=False)
    w_hid = nc.declare_dram_parameter("W_hidden", [H, A], F32, isOutput=False)
    v = nc.declare_dram_parameter("v", [A], F32, isOutput=False)
    ctx_out = nc.declare_dram_parameter("context", [BC, E], F32, isOutput=True)
    align_out = nc.declare_dram_parameter("alignment", [L, BC], F32, isOutput=True)

    with tile.TileContext(nc) as tc:
        with tc.tile_pool(name="const", bufs=1) as const:
            ident = const.tile([P, P], F32)
            make_identity(nc, ident)
            ident_r = const.tile([P, P], F32R)
            nc.sync.dma_start(ident_r, ident.bitcast(F32R))
            ones_f32 = const.tile([1, P], F32)
            nc.gpsimd.memset(ones_f32, 1.0)

            w_sb = const.tile([P, KC, A], F32R)
            nc.sync.dma_start(
                w_sb, w_enc.rearrange("(k p) a -> p k a", p=P).bitcast(F32R)
            )

            v_row = const.tile([1, A], F32)
            nc.sync.dma_start(v_row, v[None, :])
            v_bc = const.tile([P, A], F32)



            mask_sb = const.tile([P, NLB, BC], F32)
            nc.sync.dma_start(mask_sb, mask.rearrange("(i p) b -> p i b", p=P))

            align_slab = const.tile([P, NLB, BC], F32)
            p_slab = const.tile([P, NLB, BC], F32)
            qb_sb = None  # staged below in the q-phase pool
            q_bc = [
                const.tile([P, A], F32, name=f"q_bc{b}") for b in range(BC)
            ]
            ctx_sb = const.tile([1, BC, E], F32)  # partition 0

            # ---- q = hidden @ W_hidden + b_attn  (tiny, one-time) ----
            with (
                tc.tile_pool(name="qtmp", bufs=2) as qtmp,
                tc.tile_pool(name="qps", bufs=2, space="PSUM") as qps,
                tc.tile_pool(name="qtps", bufs=2, space="PSUM") as qtps,
            ):
                for h in range(NH):
                    vb_ps = qps.tile([P, 512], F32, tag="vb", name=f"vb_ps{h}")
                    nc.tensor.matmul(
                        vb_ps,
                        lhsT=ones_f32,
                        rhs=v_row[:, h * 512 : (h + 1) * 512],
                        start=True,
                        stop=True,
                    )
                    nc.vector.tensor_copy(v_bc[:, h * 512 : (h + 1) * 512], vb_ps)
                battn4 = qtmp.tile([BC, A], F32, tag="battn")
                for b in range(BC):
                    nc.sync.dma_start(battn4[b : b + 1, :], b_attn[None, :])
                qb4 = qtmp.tile([BC, A], F32, tag="qb4")
                qb_sb = qtmp.tile([1, BC, A], F32, tag="qbsb")
                hid_sb = qtmp.tile([BC, H], F32, tag="hid")
                nc.sync.dma_start(hid_sb, hid[:, :])
                hidT = qtmp.tile([P, KC, BC], F32R, tag="hidT")
                for k in range(KC):
                    tp = qtps.tile([P, BC], F32, tag="qtp")
                    nc.tensor.transpose(
                        tp, hid_sb[:, k * P : (k + 1) * P], ident[:BC, :BC]
                    )
                    nc.vector.tensor_copy(hidT[:, k, :], tp)
                q_ps = [qps.tile([BC, 512], F32, tag="qps", name=f"q_ps{h}") for h in range(NH)]
                for k in range(KC):
                    wh = qtmp.tile([P, A], F32R, tag="wh")
                    nc.sync.dma_start(
                        wh, w_hid[k * P : (k + 1) * P, :].bitcast(F32R)
                    )
                    for h in range(NH):
                        nc.tensor.matmul(
                            q_ps[h],
                            lhsT=hidT[:, k, :],
                            rhs=wh[:, h * 512 : (h + 1) * 512],
                            start=(k == 0),
                            stop=(k == KC - 1),
                        )
                for h in range(NH):
                    sl = slice(h * 512, (h + 1) * 512)
                    nc.vector.tensor_add(qb4[:, sl], q_ps[h], battn4[:, sl])
                for b in range(BC):
                    nc.sync.dma_start(qb_sb[0:1, b, :], qb4[b : b + 1, :])
                for b in range(BC):
                    for h in range(NH):
                        qb_ps = qps.tile(
                            [P, 512], F32, tag="vb", name=f"qb_ps_{b}_{h}"
                        )
                        nc.tensor.matmul(
                            qb_ps,
                            lhsT=ones_f32,
                            rhs=qb_sb[0:1, b, h * 512 : (h + 1) * 512],
                            start=True,
                            stop=True,
                        )
                        nc.vector.tensor_copy(
                            q_bc[b][:, h * 512 : (h + 1) * 512], qb_ps
                        )

            # ---- main pipeline ----
            with (
                tc.tile_pool(name="xpool", bufs=20) as xpool,
                tc.tile_pool(name="xtpool", bufs=3) as xtpool,
                tc.tile_pool(name="thpool", bufs=2) as thpool,
                tc.tile_pool(name="scrpool", bufs=2) as scrpool,
                tc.tile_pool(name="smpool", bufs=2) as smpool,
                tc.tile_pool(name="eslab", bufs=2) as espool,
                tc.tile_pool(name="keysps", bufs=2, space="PSUM") as keysps,
                tc.tile_pool(name="tpps", bufs=2, space="PSUM") as tpps,
                tc.tile_pool(name="ctxps", bufs=2, space="PSUM") as ctxps,
                tc.tile_pool(name="tinyps", bufs=1, space="PSUM") as tinyps,
            ):
                x_tiles = {}

                def emit_block(b, i):
                    x = xpool.tile([P, E], F32R, tag="x", name=f"x_{b}_{i}")
                    nc.sync.dma_start(
                        x, enc[i * P : (i + 1) * P, b, :].bitcast(F32R)
                    )
                    x_tiles[(b, i)] = x

                    xt = xtpool.tile([P, E], F32R, tag="xt", name=f"xt_{b}_{i}")
                    for g in range(2):
                        tp = tpps.tile([P, 512], F32R, tag="tp", name=f"tp_{b}_{i}_{g}")
                        for j in range(4):
                            k = g * 4 + j
                            nc.tensor.transpose(
                                tp[:, j * P : (j + 1) * P],
                                x[:, k * P : (k + 1) * P],
                                ident_r,
                            )
                        nc.scalar.copy(xt[:, g * 512 : (g + 1) * 512], tp)

                    th = thpool.tile([P, E], F32, tag="th", name=f"th_{b}_{i}")
                    kp = [
                        keysps.tile([P, 512], F32, tag="kp", name=f"kp_{b}_{i}_{h}")
                        for h in range(NH)
                    ]
                    for k in range(KC):
                        for h in range(NH):
                            nc.tensor.matmul(
                                kp[h],
                                lhsT=xt[:, k * P : (k + 1) * P],
                                rhs=w_sb[:, k, h * 512 : (h + 1) * 512],
                                start=(k == 0),
                                stop=(k == KC - 1),
                            )
                    thp = thpool.tile([P, E], F32, tag="thp", name=f"thp_{b}_{i}")
                    for h in range(NH):
                        sl = slice(h * 512, (h + 1) * 512)
                        nc.vector.tensor_add(thp[:, sl], kp[h], q_bc[b][:, sl])
                        nc.scalar.activation(th[:, sl], thp[:, sl], AF.Tanh)

                    scr = scrpool.tile([P, E], F32, tag="scr", name=f"scr_{b}_{i}")
                    nc.vector.tensor_mul(scr, th, v_bc)
                    nc.vector.tensor_reduce(
                        align_slab[:, i, b : b + 1],
                        scr,
                        axis=mybir.AxisListType.X,
                        op=ALU.add,
                    )

                def emit_epilogue(b):
                    # exp (no max-shift; logits are O(10)) + per-partition sum
                    e_in = espool.tile([P, NLB], F32, tag="ein", name=f"ein_{b}")
                    nc.vector.tensor_add(e_in, align_slab[:, :, b], mask_sb[:, :, b])
                    e_sl = espool.tile([P, NLB], F32, tag="esl", name=f"esl_{b}")
                    rowsum = smpool.tile([P, 1], F32, tag="rs", name=f"rs_{b}")
                    nc.scalar.activation(e_sl, e_in, AF.Exp, accum_out=rowsum)
                    e_slr = espool.tile([P, NLB], F32R, tag="eslr", name=f"eslr_{b}")
                    nc.vector.tensor_copy(e_slr, e_sl)
                    # context matvecs on unnormalized exp weights
                    for h in range(NH):
                        cp = ctxps.tile([1, 512], F32, tag="cp", name=f"cp_{b}_{h}")
                        for i in range(NLB):
                            nc.tensor.matmul(
                                cp,
                                lhsT=e_slr[:, i : i + 1],
                                rhs=x_tiles[(b, i)][:, h * 512 : (h + 1) * 512],
                                start=(i == 0),
                                stop=(i == NLB - 1),
                            )
                        # normalization: partition-sum of rowsum via PE transpose
                        if h == 0:
                            ts = tinyps.tile([1, P], F32, tag="ts", name=f"ts_{b}")
                            nc.tensor.transpose(ts, rowsum, ident)
                            s_sb = smpool.tile([1, 1], F32, tag="s", name=f"s_{b}")
                            nc.vector.tensor_reduce(
                                s_sb, ts, axis=mybir.AxisListType.X, op=ALU.add
                            )
                            rinv = smpool.tile([1, 1], F32, tag="ri", name=f"ri_{b}")
                            nc.vector.reciprocal(rinv, s_sb)
                            rb_ps = tinyps.tile(
                                [P, 1], F32, tag="rbps", name=f"rbps_{b}"
                            )
                            nc.tensor.matmul(
                                rb_ps,
                                lhsT=ones_f32,
                                rhs=rinv,
                                start=True,
                                stop=True,
                            )
                            rinv_bc = smpool.tile(
                                [P, 1], F32, tag="rib", name=f"rib_{b}"
                            )
                            nc.vector.tensor_copy(rinv_bc, rb_ps)
                            nc.vector.tensor_scalar_mul(p_slab[:, :, b], e_sl, rinv_bc)
                        nc.vector.tensor_scalar_mul(
                            ctx_sb[0:1, b, h * 512 : (h + 1) * 512], cp, rinv
                        )

                for b in range(BC):
                    for i in range(NLB):
                        emit_block(b, i)
                        if b > 0 and i == 1:
                            emit_epilogue(b - 1)
                emit_epilogue(BC - 1)

                nc.sync.dma_start(
                    ctx_out[None, :, :], ctx_sb[0:1, :, :]
                )
                nc.sync.dma_start(
                    align_out.rearrange("(i p) b -> p i b", p=P), p_slab
                )

    if finalize:
        nc.finalize()
    return nc


_NC_CACHE = None


def _get_nc():
    global _NC_CACHE
    if _NC_CACHE is None:
        _NC_CACHE = build_nc()
    return _NC_CACHE


def _shard_inputs(inputs):
    enc = np.ascontiguousarray(np.asarray(inputs["enc_outputs"], dtype=np.float32))
    mask = np.ascontiguousarray(np.asarray(inputs["mask"], dtype=np.float32))
    hid = np.ascontiguousarray(np.asarray(inputs["hidden_state"], dtype=np.float32))
    w_enc = np.ascontiguousarray(np.asarray(inputs["W_enc"], dtype=np.float32))
    b_attn = np.ascontiguousarray(np.asarray(inputs["b_attn"], dtype=np.float32))
    w_hid = np.ascontiguousarray(np.asarray(inputs["W_hidden"], dtype=np.float32))
    v = np.ascontiguousarray(np.asarray(inputs["v"], dtype=np.float32))
    in_maps = []
    for c in range(NCORES):
        bs = slice(BC * c, BC * (c + 1))
        in_maps.append(
            {
                "enc_outputs": np.ascontiguousarray(enc[:, bs, :]),
                "mask": np.ascontiguousarray(mask[:, bs]),
                "hidden_state": np.ascontiguousarray(hid[bs, :]),
                "W_enc": w_enc,
                "b_attn": b_attn,
                "W_hidden": w_hid,
                "v": v,
            }
        )
    return in_maps


def run_spmd(inputs, trace=False):
    """Returns (context, alignment), BassKernelResults."""
    in_maps = _shard_inputs(inputs)
    res = run_bass_kernel_spmd(
        _get_nc(), in_maps, list(range(NCORES)), trace=trace
    )
    ctx = np.concatenate(
        [np.asarray(res.results[c]["context"]) for c in range(NCORES)], axis=0
    )
    align = np.concatenate(
        [np.asarray(res.results[c]["alignment"]) for c in range(NCORES)], axis=1
    )
    return (ctx, align), res


def kernel(**inputs):
    out, _ = run_spmd(inputs, trace=False)
    return out
